# revision 46
# baseline (speedup 1.0000x reference)
"""CharEmb kernel for Trainium2 (8 NeuronCores, batch-sharded).

Computation (per word of 32 chars):
  emb = table[ids]                  # [32 chars, 64] gathered
  x[i, j] = emb[i//2, 32*(i%2)+j]   # raw-buffer reshape [64, 32]
  y[f, t] = sum_{i,k} x[i, t+k] * w[f, i, k]   (valid conv, K=3)
  out[f] = max_t y[f, t] + b[f]

v3: hybrid pair-table SWDGE gather + PE one-hot lookup.
  - The SWDGE gather path is Q7 descriptor-gen bound (~8 ns/idx), so the
    embedding lookup is split across two engines:
      * GpSimd: pair-table gather.  ptab[v1*101+v2] = [table[v1];table[v2]]
        (256-B bf16 rows); one element covers char c of words (2P, 2P+1).
        Handles blocks 0..BS-1 of each 16-block (128-word) group.
      * TensorE: one-hot lookup for blocks BS..15.  Per what-column
        (4 words), stationary = host-built one-hot [101, 128] and moving =
        the bf16 table [101, 64]: out[32j+c, u] = table[ids[w(j),c], u] in
        PSUM, copied to the gather tile by the (idle) Scalar engine.
  - Layout: element/col (blk, j, c) -> partition 32j+c.  A 128-partition
    block = 8 words; free dim 128*blk + 64*o + u, what = 2*blk+o.
  - conv: 6 accumulating bf16 K=32 matmuls per word-slot, row-tiled via
    tile_position, one single-bank PSUM tile per slot (pool of 6).
  - maxpool: per-(chunk, slot) tensor_reduce(max) over the 30 t cols.
"""

import sys
from contextlib import ExitStack

import numpy as np

if "/opt/trn_rl_repo" not in sys.path:
    sys.path.insert(0, "/opt/trn_rl_repo")

import concourse.bass as bass
import concourse.tile as tile
from concourse import bacc, mybir
from concourse.bass_utils import run_bass_kernel_spmd

# Problem constants (hardcoded per spec)
B, S, C = 32, 512, 32
V, E = 101, 64
F, K = 128, 3
T = C - K + 1  # 30 valid conv positions
NCORES = 8
WORDS = (B * S) // NCORES  # 2048 words per core

GATHER_WORDS = 128  # words per gather group
NGROUPS = WORDS // GATHER_WORDS  # 16
CHUNK_WORDS = 64  # words per conv/psum chunk
NCHUNKS = WORDS // CHUNK_WORDS  # 32

# blocks (of 16) per group fetched via SWDGE; rest via PE one-hot.
# tapered at the end so the final groups' convs start sooner.
BS_LIST = [4] * 14 + [3, 2]
PE_COLS = [(16 - b) * 2 for b in BS_LIST]  # one-hot what-columns per group
GCOLS_LIST = [8 * b for b in BS_LIST]  # idx columns per group
IDXC_OFF = [sum(GCOLS_LIST[:g]) for g in range(NGROUPS + 1)]
OHC_OFF = [sum(PE_COLS[:g]) * 128 for g in range(NGROUPS + 1)]

f32 = mybir.dt.float32
f8e4 = mybir.dt.float8e4
bf16 = mybir.dt.bfloat16
i16 = mybir.dt.int16


def build_kernel(num_devices=NCORES, add_bias=True):
    words = WORDS

    nc = bacc.Bacc(
        "TRN2",
        target_bir_lowering=False,
        debug=False,
        enable_asserts=False,
        num_devices=num_devices,
    )

    idx_d = nc.dram_tensor("idx", [128, IDXC_OFF[-1]], i16, kind="ExternalInput")
    tab_d = nc.dram_tensor("tab", [V * V, 2 * E], bf16, kind="ExternalInput")
    tabs_d = nc.dram_tensor("tabs", [V, E], bf16, kind="ExternalInput")
    oh_d = nc.dram_tensor("oh", [128 * OHC_OFF[-1]], f8e4, kind="ExternalInput")
    w_d = nc.dram_tensor("wmat", [128, 6 * 128], bf16, kind="ExternalInput")
    b_d = nc.dram_tensor("bias", [128, 1], f32, kind="ExternalInput")
    # f-major output: out[f, col] with col = 64c + 16j + what
    #   -> word 64c + 8*(what//2) + 2j + what%2
    out_d = nc.dram_tensor("out", [128, words], f32, kind="ExternalOutput")

    with tile.TileContext(nc) as tc, ExitStack() as ctx:
        const_pool = ctx.enter_context(tc.tile_pool(name="const", bufs=1))
        g_pool = ctx.enter_context(tc.tile_pool(name="gath", bufs=4))
        pA_pool = ctx.enter_context(tc.tile_pool(name="psA", bufs=6, space="PSUM"))
        pB_pool = ctx.enter_context(tc.tile_pool(name="psB", bufs=2, space="PSUM"))

        idx_sb = const_pool.tile([128, IDXC_OFF[-1]], i16)
        tabs_sb = const_pool.tile([128, E], bf16)
        oh_sb = const_pool.tile([128, OHC_OFF[-1]], f8e4)
        w_sb = const_pool.tile([128, 6 * 128], bf16)
        b_sb = const_pool.tile([128, 1], f32)
        obuf = const_pool.tile([128, words], f32)

        # per-group idx slices on Sync: tiny transfers, so the first gather
        # starts as early as possible
        for gi in range(NGROUPS):
            nc.sync.dma_start(
                idx_sb[:, IDXC_OFF[gi]:IDXC_OFF[gi + 1]],
                idx_d.ap()[:, IDXC_OFF[gi]:IDXC_OFF[gi + 1]],
            )
        nc.sync.dma_start(tabs_sb[0:V, :], tabs_d.ap())
        nc.sync.dma_start(w_sb[:], w_d.ap())
        nc.sync.dma_start(b_sb[:], b_d.ap())
        # one-hot preload on the Scalar HWDGE queue ([128, N] shapes spray
        # across all 16 DMA engines; keeps Sync free for idx/out)
        for gi in range(NGROUPS):
            ohc = PE_COLS[gi] * 128
            nc.scalar.dma_start(
                oh_sb[:, OHC_OFF[gi]:OHC_OFF[gi + 1]],
                oh_d.ap()[128 * OHC_OFF[gi]:128 * OHC_OFF[gi + 1]]
                .rearrange("(p c) -> p c", c=ohc),
            )

        for gi in range(NGROUPS):
            bs = BS_LIST[gi]
            pe_cols = PE_COLS[gi]
            gidx = 128 * bs
            # --- SWDGE gather: pair-embeddings for blocks 0..bs-1 ---
            g = g_pool.tile([128, 16 * 2 * E], bf16)  # 16 blocks x 256B
            g_r = g[:, 0:bs * 2 * E].rearrange("p (b e) -> p b e", e=2 * E)
            nc.gpsimd.dma_gather(
                out_ap=g_r,
                in_ap=tab_d.ap(),
                idxs_ap=idx_sb[:, IDXC_OFF[gi]:IDXC_OFF[gi + 1]],
                num_idxs=gidx,
                num_idxs_reg=gidx,
                elem_size=2 * E,
                single_packet=False,
            )
            # unified view, 64-wide what-columns: [128, 32 what, 64]
            g_w = g[:].rearrange("p (x j) -> p x j", j=E)

            # --- PE one-hot lookup for blocks bs..15 (pe_cols what-cols) ---
            n_banks = (pe_cols + 7) // 8
            pbs = [
                pB_pool.tile([128, 512], f32, name="pb")
                for _ in range(n_banks)
            ]
            for b in range(pe_cols):
                pb = pbs[b // 8]
                nc.tensor.matmul(
                    pb[:, (b % 8) * E:(b % 8) * E + E],
                    oh_sb[0:V, OHC_OFF[gi] + b * 128:OHC_OFF[gi] + (b + 1) * 128],
                    tabs_sb[0:V, :],
                    start=True,
                    stop=True,
                )
            # one contiguous PSUM->SBUF copy per bank (blocks 8k..8k+7 land
            # on consecutive what-columns of g)
            for k in range(n_banks):
                nb = min(8, pe_cols - 8 * k)
                x0 = 2 * bs + 8 * k
                nc.scalar.copy(
                    g[:, E * x0:E * (x0 + nb)], pbs[k][:, 0:nb * E]
                )

            # --- conv + maxpool, two 64-word chunks per group ---
            # ci=1 first: at bs<=4 its 8 blocks are all PE-produced, so its
            # convs don't wait for the gather at all
            for ci in (1, 0):
                c = gi * 2 + ci
                pts = [
                    pA_pool.tile([128, 512], f32, name="pa")
                    for _ in range(4)
                ]
                for hk in range(6):
                    h, k = divmod(hk, 3)
                    j0 = 32 * h + k
                    for s in range(4):
                        out_ap = (
                            pts[s][:, 0:16 * T]
                            .rearrange("f (w t) -> f w t", t=T)
                        )
                        rhs = g_w[32 * s:32 * s + 32,
                                  ci * 16:(ci + 1) * 16, j0:j0 + T]
                        lhsT = w_sb[32 * s:32 * s + 32, 128 * hk:128 * hk + 128]
                        nc.tensor.matmul(
                            out_ap,
                            lhsT,
                            rhs,
                            start=(hk == 0),
                            stop=(hk == 5),
                            tile_position=(32 * s, 0),
                            skip_group_check=True,
                        )

                for s in range(4):
                    p_v = (
                        pts[s][:, 0:16 * T]
                        .rearrange("f (w t) -> f w t", t=T)
                    )
                    o_v = obuf[:, c * CHUNK_WORDS + 16 * s:
                               c * CHUNK_WORDS + 16 * s + 16]
                    nc.vector.tensor_reduce(
                        o_v, p_v, axis=mybir.AxisListType.X,
                        op=mybir.AluOpType.max,
                    )

            # --- bias + store per 4 groups (512 words), overlapped ---
            if (gi + 1) % 4 == 0:
                q0 = (gi + 1 - 4) * GATHER_WORDS
                q1 = (gi + 1) * GATHER_WORDS
                if add_bias:
                    nc.scalar.add(
                        obuf[:, q0:q1], obuf[:, q0:q1], b_sb[:, 0:1]
                    )
                nc.sync.dma_start(out_d.ap()[:, q0:q1], obuf[:, q0:q1])

    nc.compile()
    return nc


def host_prep(char_ids, emb_table, conv_w, conv_b, num_devices=NCORES):
    """Build per-core input maps from full inputs."""
    from ml_dtypes import bfloat16 as np_bf16
    from ml_dtypes import float8_e4m3 as np_f8e4

    words = WORDS
    char_ids = np.asarray(char_ids)
    emb_table = np.asarray(emb_table, dtype=np.float32)
    conv_w = np.asarray(conv_w, dtype=np.float32)
    conv_b = np.asarray(conv_b, dtype=np.float32)

    tab_bf = emb_table.astype(np_bf16)  # [101, 64]
    # pair table: ptab[v1*101+v2] = [table[v1] ; table[v2]]
    ptab = np.zeros((V, V, 2, E), dtype=np_bf16)
    ptab[:, :, 0, :] = tab_bf[:, None, :]
    ptab[:, :, 1, :] = tab_bf[None, :, :]
    ptab = np.ascontiguousarray(ptab.reshape(V * V, 2 * E))

    ids_flat = char_ids.reshape(-1, C).astype(np.int32)  # [16384, 32]

    # stationary weights: wmat[32s+p, 128*(3h+k) + f] = conv_w[f, 2p+h, k]
    wmat = np.zeros((128, 6 * 128), dtype=np.float32)
    for h in range(2):
        for k in range(3):
            hk = 3 * h + k
            w_pf = conv_w[:, h::2, k].T  # [32 p, 128 f]
            wmat[:, 128 * hk:128 * (hk + 1)] = np.tile(w_pf, (4, 1))
    wmat = wmat.astype(np_bf16)

    bias = conv_b.reshape(128, 1)

    in_maps = []
    for j in range(num_devices):
        ids_core = ids_flat[j * words:(j + 1) * words]  # [words, 32]
        ids_g = ids_core.reshape(NGROUPS, 16, 4, 2, C)  # [g, blk, jh, o, c]

        flat_parts = []
        oh_parts = []
        for g in range(NGROUPS):
            bs = BS_LIST[g]
            # SWDGE pair stream for blocks 0..bs-1:
            # element m (group-local) = 128*blk + 32*jh + c
            pid = ids_g[g, :bs, :, 0, :] * V + ids_g[g, :bs, :, 1, :]
            flat_parts.append(pid.reshape(-1))
            # one-hot for PE blocks bs..15 (col = blockg*128 + 32*jh + c,
            # blockg = (blk-bs)*2 + o), padded to 128 rows so the DMA
            # sprays across all 16 engines
            ids_pe = ids_g[g, bs:, :, :, :].transpose(0, 2, 1, 3)
            ids_pe = ids_pe.reshape(-1)  # [(blk, o, jh, c)]
            ohg = np.zeros((128, ids_pe.size), dtype=np_f8e4)
            ohg[:V, :] = (
                np.arange(V, dtype=np.int32)[:, None] == ids_pe[None, :]
            ).astype(np_f8e4)
            oh_parts.append(ohg.reshape(-1))
        flat = np.concatenate(flat_parts).astype(np.int16)
        wrapped = flat.reshape(-1, 16).T.copy()  # [16, ncols]
        idx = np.tile(wrapped, (8, 1))  # replicate to 128 partitions
        oh = np.ascontiguousarray(np.concatenate(oh_parts))

        in_maps.append(
            {
                "idx": np.ascontiguousarray(idx),
                "tab": ptab,
                "tabs": tab_bf,
                "oh": oh,
                "wmat": wmat,
                "bias": bias,
            }
        )
    return in_maps


def _ensure_ntff_hook():
    """The agent image's antenv lacks axon_hooks; shim it and install the
    ctypes NTFF profiling hook so trace=True yields HW exec times."""
    import types

    if "antenv.axon_hooks" in sys.modules:
        return
    mod = types.ModuleType("antenv.axon_hooks")
    _hook = [None]
    mod.get_axon_ntff_profile_hook = lambda: _hook[0]
    mod.set_axon_ntff_profile_hook = lambda h: _hook.__setitem__(0, h)
    sys.modules["antenv.axon_hooks"] = mod
    try:
        import antenv

        antenv.axon_hooks = mod
        from trn_agent_boot.trn_boot import _ntff_profile_via_ctypes

        hook = _ntff_profile_via_ctypes("/opt/axon/libaxon_pjrt.so")
        mod.set_axon_ntff_profile_hook(hook)
    except Exception as e:  # degrade to no-trace
        print(f"ntff hook install failed: {e}", file=sys.stderr)


_NC_CACHE = {}


def _get_nc():
    if "nc" not in _NC_CACHE:
        _NC_CACHE["nc"] = build_kernel()
    return _NC_CACHE["nc"]


def unscramble_out(raw, words=WORDS):
    """[128 f, words], col = 64c + 16j + what, what = 2*blk + o
    -> word 64c + 8*blk + 2*j + o."""
    nchunks = words // CHUNK_WORDS
    o = raw.reshape(128, nchunks, 4, 8, 2)  # [f, c, j, blk, o]
    o = o.transpose(1, 3, 2, 4, 0)  # [c, blk, j, o, f]
    return np.ascontiguousarray(o.reshape(words, 128))


def kernel(char_ids, emb_table, conv_w, conv_b, trace=False):
    if trace:
        _ensure_ntff_hook()
    nc = _get_nc()
    in_maps = host_prep(char_ids, emb_table, conv_w, conv_b)
    res = run_bass_kernel_spmd(
        nc, in_maps, core_ids=list(range(NCORES)), trace=trace
    )
    outs = [unscramble_out(res.results[j]["out"]) for j in range(NCORES)]
    full = np.concatenate(outs, axis=0).reshape(B, S, F).astype(np.float32)
    if trace:
        return full, res
    return full
